# revision 32
# baseline (speedup 1.0000x reference)
"""Multi-head self-attention (B=2, S=2048, D=1024, H=16, causal) on 8 trn2 cores.

Sharding: core c = (batch b=c//4, head-group g=c%4 of 4 heads = dims
256g:256g+256). Column-parallel QKV, row-parallel O; each core returns a
partial [2048, 1024] output for its batch; host sums 4 partials per batch.

Per-core kernel (all matmul inputs bf16):
  - projections from xt [1024, 2048] bf16: QT/KT [128, 2, 2048] bf16;
    V computed directly in token-major layout vg[128, 16, 4, 65] with a
    ones column per head so AV also produces the softmax denominator.
  - attention in transposed-score layout: scoresT[k, q] = K @ Q^T tiles,
    exp on ACT (scale 1/8 fused) to bf16, causal staircase skips invalid
    columns, triangular mask on diagonal blocks only. Lag-2 AV pipeline.
  - software pipeline: only the first token-chunk of Q/K and V tiles 0-3
    are projected up front; the remaining projection chains run as PE
    fillers inside the attention j-loops (which are paced by the exp
    stream on ACT), as do the deferred normalize + O-projection units.
  - attention runs two sweeps of 2 heads per q-chunk so the exp covers
    a [128, 2, 512] two-bank PSUM pair in one ACT instruction (halves
    exp instruction count and ACT semaphore waits).
  - normalize: raw AV copy to SBUF (frees PSUM), denominator broadcast
    via f32r ones outer-product matmul, reciprocal_approx_fast on DVE
    (no Ln -> no ACT table switches), multiply to bf16 pairs; odd heads
    DMA-shifted to partitions 64:128 so O contracts 128 dims per pair.
  - O projection: 2 chained matmuls per 512-col chunk, single output
    DMA per 128-token tile; the qc=3 normalize is chunked by token tile
    so the tail O units start as early as possible.
"""

import os
import numpy as np
from contextlib import ExitStack

import ml_dtypes

import concourse.bass as bass
import concourse.tile as tile
from concourse import bacc, mybir
from concourse.bass_utils import run_bass_kernel_spmd

F32R = mybir.dt.float32r
F32 = mybir.dt.float32
BF16 = mybir.dt.bfloat16
EXP = mybir.ActivationFunctionType.Exp
COPY = mybir.ActivationFunctionType.Copy

B, S, D = 2, 2048, 1024
NCORES = 8
SCALE = 0.125         # 1/sqrt(64)
NH = 4                # heads per core

_BUILT = None
LAST_RESULTS = None


def _build():
    nc = bacc.Bacc("TRN2", target_bir_lowering=False, debug=False,
                   num_devices=NCORES)
    # xt packed [p, tc, k, c]: xt[p, tc, k, c] = x[b][512tc+c, 128k+p]
    xt_d = nc.dram_tensor("xt", [128, 4, 8, 512], BF16,
                          kind="ExternalInput").ap()
    wq_d = nc.dram_tensor("wq", [128, 2048], BF16, kind="ExternalInput").ap()
    wk_d = nc.dram_tensor("wk", [128, 2048], BF16, kind="ExternalInput").ap()
    wvt_d = nc.dram_tensor("wvt", [128, 2048], BF16, kind="ExternalInput").ap()
    wo_d = nc.dram_tensor("wo", [128, 2048], BF16, kind="ExternalInput").ap()
    tri_d = nc.dram_tensor("tri", [128, 128], BF16, kind="ExternalInput").ap()
    onesb_d = nc.dram_tensor("onesb", [128, 64], BF16,
                             kind="ExternalInput").ap()
    onesr_d = nc.dram_tensor("onesr", [1, 64], F32R, kind="ExternalInput").ap()
    out_d = nc.dram_tensor("out", [S, D], BF16, kind="ExternalOutput").ap()

    with tile.TileContext(nc) as tc, ExitStack() as ctx:
        consts = ctx.enter_context(tc.tile_pool(name="consts", bufs=1))
        sb = ctx.enter_context(tc.tile_pool(name="sb", bufs=1))
        ps = ctx.enter_context(tc.tile_pool(name="ps", bufs=1, space="PSUM"))

        # ---- DMA program: weights on the ACT hwdge queue, x on sync ----
        # (halves so the first projection chains start sooner)
        wq_t = consts.tile([128, 2048], BF16, tag="wq")
        nc.scalar.dma_start(wq_t[:, 0:1024], wq_d[:, 0:1024])
        nc.scalar.dma_start(wq_t[:, 1024:2048], wq_d[:, 1024:2048])
        wk_t = consts.tile([128, 2048], BF16, tag="wk")
        nc.scalar.dma_start(wk_t[:, 0:1024], wk_d[:, 0:1024])
        nc.scalar.dma_start(wk_t[:, 1024:2048], wk_d[:, 1024:2048])
        wvt_t = consts.tile([128, 2048], BF16, tag="wvt")
        nc.scalar.dma_start(wvt_t, wvt_d)
        wo_t = consts.tile([128, 2048], BF16, tag="wo")
        nc.scalar.dma_start(wo_t, wo_d)
        # x: one big contiguous DMA per token-chunk (tc0 split for latency)
        xts = []
        for tc2 in range(4):
            xk = sb.tile([128, 8, 512], BF16, tag="xts", bufs=4,
                         name=f"xts{tc2}")
            if tc2 == 0:
                for kq in range(4):
                    nc.sync.dma_start(xk[:, 2 * kq:2 * kq + 2, :],
                                      xt_d[:, 0, 2 * kq:2 * kq + 2, :])
            else:
                nc.sync.dma_start(xk, xt_d[:, tc2, :, :])
            xts.append(xk)
        tri_t = consts.tile([128, 128], BF16, tag="tri")
        nc.gpsimd.dma_start(tri_t, tri_d)
        onesb_t = consts.tile([128, 64], BF16, tag="onesb")
        nc.gpsimd.dma_start(onesb_t, onesb_d)
        # ones row at partition 64 (same base partition as the den row)
        onesr_t = consts.tile([65, 64], F32R, tag="onesr")
        nc.gpsimd.dma_start(onesr_t[64:65, :], onesr_d)

        qt = sb.tile([128, 2, 2048], BF16, tag="qt")
        kt = sb.tile([128, 2, 2048], BF16, tag="kt")
        vg = sb.tile([128, 16, NH, 65], BF16, tag="vg")
        # strided ones fill on the Pool engine (a strided DMA here would
        # degenerate to 2-byte SWDGE packets)
        nc.gpsimd.tensor_copy(vg[:, :, :, 64:65], onesb_t[:, 0:64])

        def qk_chain(w_t, dst, gg, tc2):
            pp = ps.tile([128, 512], F32, tag="mm", bufs=2)
            for k in range(8):
                nc.tensor.matmul(
                    pp,
                    lhsT=w_t[:, 1024 * gg + 128 * k:1024 * gg + 128 * (k + 1)],
                    rhs=xts[tc2][:, k, :],
                    start=(k == 0), stop=(k == 7), skip_group_check=True)
            nc.vector.tensor_copy(dst[:, gg, 512 * tc2:512 * (tc2 + 1)], pp)

        def v_chain(j):
            vp = ps.tile([128, 256], F32, tag="mm", bufs=2)
            for k in range(8):
                nc.tensor.matmul(
                    vp,
                    lhsT=xts[j // 4][:, k, 128 * (j % 4):128 * (j % 4 + 1)],
                    rhs=wvt_t[:, 256 * k:256 * (k + 1)],
                    start=(k == 0), stop=(k == 7), skip_group_check=True)
            nc.vector.tensor_copy(vg[:, j, :, 0:64], vp)

        # ---- deferred normalize (part1) / O projection (part2) ----
        raw_of = {}
        pair_of = {}

        def part1_norm(qc):
            raws = raw_of[qc]
            pairs = [sb.tile([128, 512], BF16, tag="pair", bufs=4,
                             name=f"pair{qc}_{p}") for p in range(2)]
            for h in range(NH):
                bc = ps.tile([64, 512], F32, tag="mm", bufs=2,
                             name=f"bc{qc}_{h}")
                nc.tensor.matmul(bc, lhsT=onesr_t[64:65, 0:64],
                                 rhs=raws[h][64:65, :],
                                 start=True, stop=True,
                                 skip_group_check=True)
                rcp = sb.tile([64, 512], F32, tag="rcp", bufs=4,
                              name=f"rcp{qc}_{h}")
                nc.vector.reciprocal_approx_fast(rcp, bc)
                if h % 2 == 0:
                    nc.vector.tensor_mul(pairs[h // 2][0:64, :],
                                         raws[h][0:64, :], rcp)
                else:
                    tmp = sb.tile([64, 512], BF16, tag="tmp", bufs=2,
                                  name=f"tmp{qc}_{h}")
                    nc.vector.tensor_mul(tmp, raws[h][0:64, :], rcp)
                    nc.scalar.dma_start(pairs[h // 2][64:128, :], tmp)
            pair_of[qc] = pairs

        def part2_unit(qc, tt):
            pairs = pair_of[qc]
            row0 = 512 * qc + 128 * tt
            ost = sb.tile([128, 1024], BF16, tag="ost", bufs=3,
                          name=f"ost{qc}_{tt}")
            for chv in range(2):
                op = ps.tile([128, 512], F32, tag="mm", bufs=2,
                             name=f"op{qc}_{tt}_{chv}")
                nc.tensor.matmul(
                    op, lhsT=pairs[0][:, 128 * tt:128 * (tt + 1)],
                    rhs=wo_t[:, 512 * chv:512 * (chv + 1)],
                    start=True, stop=False, skip_group_check=True)
                nc.tensor.matmul(
                    op, lhsT=pairs[1][:, 128 * tt:128 * (tt + 1)],
                    rhs=wo_t[:, 1024 + 512 * chv:1024 + 512 * (chv + 1)],
                    start=False, stop=True, skip_group_check=True)
                if chv == 0:
                    nc.vector.tensor_copy(ost[:, 0:512], op)
                else:
                    nc.scalar.activation(ost[:, 512:1024], op, COPY)
            nc.sync.dma_start(out_d[row0:row0 + 128, :], ost)

        # ---- attention: two sweeps of 2 heads; 1024-wide exp per j ----
        def attention(qc, fillers):
            njt = 4 * qc + 4
            nfl = len(fillers)
            for sweep in range(2):
                h0 = 2 * sweep
                avs = [ps.tile([128, 512], F32, tag="av", bufs=2,
                               name=f"avps{qc}_{sweep}_{hh}")
                       for hh in range(2)]
                pend = []

                def do_av(j, etp, avs=avs, h0=h0):
                    vs = max(0, 128 * (j - 4 * qc))
                    for hh in range(2):
                        nc.tensor.matmul(
                            avs[hh][0:65, vs:512],
                            lhsT=vg[:, j, h0 + hh, 0:65],
                            rhs=etp[:, hh, vs:512],
                            start=(j == 0), stop=(j == njt - 1),
                            skip_group_check=True)

                for j in range(njt):
                    vs = max(0, 128 * (j - 4 * qc))
                    scp = ps.tile([128, 2, 512], F32, tag="mm2", bufs=2)
                    for hh in range(2):
                        h = h0 + hh
                        hp = 64 * (h % 2)
                        gg = h // 2
                        nc.tensor.matmul(
                            scp[:, hh, vs:512],
                            lhsT=kt[hp:hp + 64, gg, 128 * j:128 * (j + 1)],
                            rhs=qt[hp:hp + 64, gg,
                                   512 * qc + vs:512 * (qc + 1)],
                            start=True, stop=True, skip_group_check=True)
                    etp = sb.tile([128, 2, 512], BF16, tag=f"et{sweep}",
                                  bufs=4)
                    nc.scalar.activation(etp[:, :, vs:512], scp[:, :, vs:512],
                                         EXP, scale=SCALE)
                    if j >= 4 * qc:
                        for hh in range(2):
                            nc.vector.tensor_mul(etp[:, hh, vs:vs + 128],
                                                 etp[:, hh, vs:vs + 128],
                                                 tri_t)
                    pend.append((j, etp))
                    if len(pend) > 2:   # lag-2 AV pipeline
                        do_av(*pend.pop(0))
                    slot = sweep * njt + j
                    k0 = nfl * slot // (2 * njt)
                    k1 = nfl * (slot + 1) // (2 * njt)
                    for k in range(k0, k1):
                        fillers[k]()
                for args in pend:
                    do_av(*args)
                # drain AV psum at the sweep boundary (frees the av banks)
                for hh in range(2):
                    raw = sb.tile([65, 512], F32R, tag="raw", bufs=8,
                                  name=f"raw{qc}_{h0 + hh}")
                    if qc == 3 and sweep == 1:
                        nc.scalar.activation(raw, avs[hh][0:65, :], COPY)
                    else:
                        nc.vector.tensor_copy(raw, avs[hh][0:65, :])
                    raw_of.setdefault(qc, []).append(raw)

        # ---- schedule ----
        # pre-phase: just enough projections for attention(0)
        qk_chain(wq_t, qt, 0, 0)
        qk_chain(wq_t, qt, 1, 0)
        qk_chain(wk_t, kt, 0, 0)
        qk_chain(wk_t, kt, 1, 0)
        for j in range(4):
            v_chain(j)

        attention(0, [
            lambda: qk_chain(wk_t, kt, 0, 1),
            lambda: qk_chain(wk_t, kt, 1, 1),
            lambda: qk_chain(wq_t, qt, 0, 1),
            lambda: qk_chain(wq_t, qt, 1, 1),
            lambda: v_chain(4),
            lambda: v_chain(5),
        ])
        attention(1, [
            lambda: v_chain(6),
            lambda: v_chain(7),
            lambda: qk_chain(wk_t, kt, 0, 2),
            lambda: qk_chain(wk_t, kt, 1, 2),
            lambda: part1_norm(0),
            lambda: qk_chain(wq_t, qt, 0, 2),
            lambda: qk_chain(wq_t, qt, 1, 2),
            lambda: v_chain(8),
            lambda: part2_unit(0, 0),
            lambda: part2_unit(0, 1),
            lambda: part2_unit(0, 2),
        ])
        attention(2, [
            lambda: v_chain(9),
            lambda: v_chain(10),
            lambda: v_chain(11),
            lambda: qk_chain(wk_t, kt, 0, 3),
            lambda: qk_chain(wk_t, kt, 1, 3),
            lambda: part2_unit(0, 3),
            lambda: part1_norm(1),
            lambda: qk_chain(wq_t, qt, 0, 3),
            lambda: qk_chain(wq_t, qt, 1, 3),
            lambda: part2_unit(1, 0),
            lambda: part2_unit(1, 1),
            lambda: v_chain(12),
        ])
        attention(3, [
            lambda: v_chain(13),
            lambda: v_chain(14),
            lambda: v_chain(15),
            lambda: part2_unit(1, 2),
            lambda: part2_unit(1, 3),
            lambda: part1_norm(2),
            lambda: part2_unit(2, 0),
            lambda: part2_unit(2, 1),
            lambda: part2_unit(2, 2),
            lambda: part2_unit(2, 3),
        ])
        # tail: normalize qc=3 in 128-column chunks so each O-projection
        # token tile starts as soon as its slice of the pairs is ready
        raws = raw_of[3]
        pairs = [sb.tile([128, 512], BF16, tag="pair", bufs=4,
                         name=f"pair3_{p}") for p in range(2)]
        rcps = []
        for h in range(NH):
            bc = ps.tile([64, 512], F32, tag="mm", bufs=2, name=f"bc3_{h}")
            nc.tensor.matmul(bc, lhsT=onesr_t[64:65, 0:64],
                             rhs=raws[h][64:65, :],
                             start=True, stop=True, skip_group_check=True)
            rcp = sb.tile([64, 512], F32, tag="rcp", bufs=4,
                          name=f"rcp3_{h}")
            nc.vector.reciprocal_approx_fast(rcp, bc)
            rcps.append(rcp)
        pair_of[3] = pairs
        for tt in range(4):
            cs = slice(128 * tt, 128 * (tt + 1))
            for h in range(NH):
                if h % 2 == 0:
                    nc.vector.tensor_mul(pairs[h // 2][0:64, cs],
                                         raws[h][0:64, cs], rcps[h][:, cs])
                else:
                    tmp = sb.tile([64, 128], BF16, tag="tmp3", bufs=4,
                                  name=f"tmp3_{h}_{tt}")
                    nc.vector.tensor_mul(tmp, raws[h][0:64, cs],
                                         rcps[h][:, cs])
                    nc.scalar.dma_start(pairs[h // 2][64:128, cs], tmp)
            part2_unit(3, tt)
    nc.compile()
    return nc


def _get_built():
    global _BUILT
    if _BUILT is None:
        _BUILT = _build()
    return _BUILT


def _host_inputs(x, q_proj, k_proj, v_proj, o_proj):
    bf = ml_dtypes.bfloat16
    # [p, tc, k, c] = x[b][512tc+c, 128k+p]
    xt = [np.ascontiguousarray(
        x[b].T.reshape(8, 128, 4, 512).transpose(1, 2, 0, 3).astype(bf))
        for b in range(B)]
    tri = np.triu(np.ones((128, 128), dtype=np.float32)).astype(bf)
    onesb = np.ones((128, 64), dtype=np.float32).astype(bf)
    onesr = np.ones((1, 64), dtype=np.float32)

    def wqk(w, g):
        # [fp, 1024*gg + 128*k + m] = w[256g+128gg+m, 128k+fp]
        a = w[256 * g:256 * (g + 1)].reshape(2, 128, 8, 128)
        return np.ascontiguousarray(
            a.transpose(3, 0, 2, 1).reshape(128, 2048).astype(bf))

    def wvt(w, g):
        # [fp, 256*k + vd] = w[256g+vd, 128k+fp]
        a = w[256 * g:256 * (g + 1)].reshape(256, 8, 128)
        return np.ascontiguousarray(
            a.transpose(2, 1, 0).reshape(128, 2048).astype(bf))

    def wo_s(w, g):
        # [dp, 1024*dd + o] = o_proj[o, 256g+128dd+dp]
        a = w[:, 256 * g:256 * (g + 1)].reshape(1024, 2, 128)
        return np.ascontiguousarray(
            a.transpose(2, 1, 0).reshape(128, 2048).astype(bf))

    in_maps = []
    for c in range(NCORES):
        b, g = c // 4, c % 4
        in_maps.append(dict(
            xt=xt[b], wq=wqk(q_proj, g), wk=wqk(k_proj, g),
            wvt=wvt(v_proj, g), wo=wo_s(o_proj, g), tri=tri,
            onesb=onesb, onesr=onesr))
    return in_maps


def kernel(**inputs):
    x = np.asarray(inputs["x"], dtype=np.float32)
    q_proj = np.asarray(inputs["q_proj"], dtype=np.float32)
    k_proj = np.asarray(inputs["k_proj"], dtype=np.float32)
    v_proj = np.asarray(inputs["v_proj"], dtype=np.float32)
    o_proj = np.asarray(inputs["o_proj"], dtype=np.float32)

    in_maps = _host_inputs(x, q_proj, k_proj, v_proj, o_proj)
    nc = _get_built()
    global LAST_RESULTS
    LAST_RESULTS = run_bass_kernel_spmd(
        nc, in_maps, core_ids=list(range(NCORES)),
        trace=bool(os.environ.get("KERNEL_TRACE")))
    out = np.zeros((B, S, D), dtype=np.float32)
    for c in range(NCORES):
        out[c // 4] += np.asarray(
            LAST_RESULTS.results[c]["out"]).astype(np.float32)
    return out


# revision 33
# speedup vs baseline: 1.0064x; 1.0064x over previous
"""Multi-head self-attention (B=2, S=2048, D=1024, H=16, causal) on 8 trn2 cores.

Sharding: core c = (batch b=c//4, head-group g=c%4 of 4 heads = dims
256g:256g+256). Column-parallel QKV, row-parallel O; each core returns a
partial [2048, 1024] output for its batch; host sums 4 partials per batch.

Per-core kernel (all matmul inputs bf16):
  - projections from xt [1024, 2048] bf16: QT/KT [128, 2, 2048] bf16;
    V computed directly in token-major layout vg[128, 16, 4, 65] with a
    ones column per head so AV also produces the softmax denominator.
  - attention in transposed-score layout: scoresT[k, q] = K @ Q^T tiles,
    exp on ACT (scale 1/8 fused) to bf16, causal staircase skips invalid
    columns, triangular mask on diagonal blocks only. Lag-2 AV pipeline.
  - software pipeline: only the first token-chunk of Q/K and V tiles 0-3
    are projected up front; the remaining projection chains run as PE
    fillers inside the attention j-loops (which are paced by the exp
    stream on ACT), as do the deferred normalize + O-projection units.
  - attention runs two sweeps of 2 heads per q-chunk so the exp covers
    a [128, 2, 512] two-bank PSUM pair in one ACT instruction (halves
    exp instruction count and ACT semaphore waits).
  - normalize: raw AV copy to SBUF (frees PSUM), denominator broadcast
    via f32r ones outer-product matmul, reciprocal_approx_fast on DVE
    (no Ln -> no ACT table switches), multiply to bf16 pairs; odd heads
    DMA-shifted to partitions 64:128 so O contracts 128 dims per pair.
  - O projection: 2 chained matmuls per 512-col chunk, single output
    DMA per 128-token tile; the qc=3 normalize is chunked by token tile
    so the tail O units start as early as possible.
"""

import os
import numpy as np
from contextlib import ExitStack

import ml_dtypes

import concourse.bass as bass
import concourse.tile as tile
from concourse import bacc, mybir
from concourse.bass_utils import run_bass_kernel_spmd

F32R = mybir.dt.float32r
F32 = mybir.dt.float32
BF16 = mybir.dt.bfloat16
EXP = mybir.ActivationFunctionType.Exp
COPY = mybir.ActivationFunctionType.Copy

B, S, D = 2, 2048, 1024
NCORES = 8
SCALE = 0.125         # 1/sqrt(64)
NH = 4                # heads per core

_BUILT = None
LAST_RESULTS = None


def _build():
    nc = bacc.Bacc("TRN2", target_bir_lowering=False, debug=False,
                   num_devices=NCORES)
    # xt packed [p, tc, k, c]: xt[p, tc, k, c] = x[b][512tc+c, 128k+p]
    xt_d = nc.dram_tensor("xt", [128, 4, 8, 512], BF16,
                          kind="ExternalInput").ap()
    wq_d = nc.dram_tensor("wq", [128, 2048], BF16, kind="ExternalInput").ap()
    wk_d = nc.dram_tensor("wk", [128, 2048], BF16, kind="ExternalInput").ap()
    wvt_d = nc.dram_tensor("wvt", [128, 2048], BF16, kind="ExternalInput").ap()
    wo_d = nc.dram_tensor("wo", [128, 2048], BF16, kind="ExternalInput").ap()
    tri_d = nc.dram_tensor("tri", [128, 128], BF16, kind="ExternalInput").ap()
    onesb_d = nc.dram_tensor("onesb", [128, 64], BF16,
                             kind="ExternalInput").ap()
    onesr_d = nc.dram_tensor("onesr", [1, 64], F32R, kind="ExternalInput").ap()
    out_d = nc.dram_tensor("out", [S, D], BF16, kind="ExternalOutput").ap()

    with tile.TileContext(nc) as tc, ExitStack() as ctx:
        consts = ctx.enter_context(tc.tile_pool(name="consts", bufs=1))
        sb = ctx.enter_context(tc.tile_pool(name="sb", bufs=1))
        ps = ctx.enter_context(tc.tile_pool(name="ps", bufs=1, space="PSUM"))

        # ---- DMA program: weights on the ACT hwdge queue, x on sync ----
        # (halves so the first projection chains start sooner)
        wq_t = consts.tile([128, 2048], BF16, tag="wq")
        nc.scalar.dma_start(wq_t[:, 0:256], wq_d[:, 0:256])
        nc.scalar.dma_start(wq_t[:, 256:1024], wq_d[:, 256:1024])
        nc.scalar.dma_start(wq_t[:, 1024:2048], wq_d[:, 1024:2048])
        wk_t = consts.tile([128, 2048], BF16, tag="wk")
        nc.scalar.dma_start(wk_t[:, 0:1024], wk_d[:, 0:1024])
        nc.scalar.dma_start(wk_t[:, 1024:2048], wk_d[:, 1024:2048])
        wvt_t = consts.tile([128, 2048], BF16, tag="wvt")
        nc.scalar.dma_start(wvt_t, wvt_d)
        wo_t = consts.tile([128, 2048], BF16, tag="wo")
        nc.scalar.dma_start(wo_t, wo_d)
        # x: one big contiguous DMA per token-chunk (tc0 split for latency)
        xts = []
        for tc2 in range(4):
            xk = sb.tile([128, 8, 512], BF16, tag="xts", bufs=4,
                         name=f"xts{tc2}")
            if tc2 == 0:
                for kq in range(4):
                    nc.sync.dma_start(xk[:, 2 * kq:2 * kq + 2, :],
                                      xt_d[:, 0, 2 * kq:2 * kq + 2, :])
            else:
                nc.sync.dma_start(xk, xt_d[:, tc2, :, :])
            xts.append(xk)
        tri_t = consts.tile([128, 128], BF16, tag="tri")
        nc.gpsimd.dma_start(tri_t, tri_d)
        onesb_t = consts.tile([128, 64], BF16, tag="onesb")
        nc.gpsimd.dma_start(onesb_t, onesb_d)
        # ones row at partition 64 (same base partition as the den row)
        onesr_t = consts.tile([65, 64], F32R, tag="onesr")
        nc.gpsimd.dma_start(onesr_t[64:65, :], onesr_d)

        qt = sb.tile([128, 2, 2048], BF16, tag="qt")
        kt = sb.tile([128, 2, 2048], BF16, tag="kt")
        vg = sb.tile([128, 16, NH, 65], BF16, tag="vg")
        # strided ones fill on the Pool engine (a strided DMA here would
        # degenerate to 2-byte SWDGE packets)
        nc.gpsimd.tensor_copy(vg[:, :, :, 64:65], onesb_t[:, 0:64])

        def qk_chain(w_t, dst, gg, tc2):
            pp = ps.tile([128, 512], F32, tag="mm", bufs=2)
            for k in range(8):
                nc.tensor.matmul(
                    pp,
                    lhsT=w_t[:, 1024 * gg + 128 * k:1024 * gg + 128 * (k + 1)],
                    rhs=xts[tc2][:, k, :],
                    start=(k == 0), stop=(k == 7), skip_group_check=True)
            nc.vector.tensor_copy(dst[:, gg, 512 * tc2:512 * (tc2 + 1)], pp)

        def v_chain(j):
            vp = ps.tile([128, 256], F32, tag="mm", bufs=2)
            for k in range(8):
                nc.tensor.matmul(
                    vp,
                    lhsT=xts[j // 4][:, k, 128 * (j % 4):128 * (j % 4 + 1)],
                    rhs=wvt_t[:, 256 * k:256 * (k + 1)],
                    start=(k == 0), stop=(k == 7), skip_group_check=True)
            nc.vector.tensor_copy(vg[:, j, :, 0:64], vp)

        # ---- deferred normalize (part1) / O projection (part2) ----
        raw_of = {}
        pair_of = {}

        def part1_norm(qc):
            raws = raw_of[qc]
            pairs = [sb.tile([128, 512], BF16, tag="pair", bufs=4,
                             name=f"pair{qc}_{p}") for p in range(2)]
            for h in range(NH):
                bc = ps.tile([64, 512], F32, tag="mm", bufs=2,
                             name=f"bc{qc}_{h}")
                nc.tensor.matmul(bc, lhsT=onesr_t[64:65, 0:64],
                                 rhs=raws[h][64:65, :],
                                 start=True, stop=True,
                                 skip_group_check=True)
                rcp = sb.tile([64, 512], F32, tag="rcp", bufs=4,
                              name=f"rcp{qc}_{h}")
                nc.vector.reciprocal_approx_fast(rcp, bc)
                if h % 2 == 0:
                    nc.vector.tensor_mul(pairs[h // 2][0:64, :],
                                         raws[h][0:64, :], rcp)
                else:
                    tmp = sb.tile([64, 512], BF16, tag="tmp", bufs=2,
                                  name=f"tmp{qc}_{h}")
                    nc.vector.tensor_mul(tmp, raws[h][0:64, :], rcp)
                    nc.scalar.dma_start(pairs[h // 2][64:128, :], tmp)
            pair_of[qc] = pairs

        def part2_unit(qc, tt):
            pairs = pair_of[qc]
            row0 = 512 * qc + 128 * tt
            ost = sb.tile([128, 1024], BF16, tag="ost", bufs=3,
                          name=f"ost{qc}_{tt}")
            for chv in range(2):
                op = ps.tile([128, 512], F32, tag="mm", bufs=2,
                             name=f"op{qc}_{tt}_{chv}")
                nc.tensor.matmul(
                    op, lhsT=pairs[0][:, 128 * tt:128 * (tt + 1)],
                    rhs=wo_t[:, 512 * chv:512 * (chv + 1)],
                    start=True, stop=False, skip_group_check=True)
                nc.tensor.matmul(
                    op, lhsT=pairs[1][:, 128 * tt:128 * (tt + 1)],
                    rhs=wo_t[:, 1024 + 512 * chv:1024 + 512 * (chv + 1)],
                    start=False, stop=True, skip_group_check=True)
                if chv == 0:
                    nc.vector.tensor_copy(ost[:, 0:512], op)
                else:
                    nc.scalar.activation(ost[:, 512:1024], op, COPY)
            nc.sync.dma_start(out_d[row0:row0 + 128, :], ost)

        # ---- attention: two sweeps of 2 heads; 1024-wide exp per j ----
        def attention(qc, fillers):
            njt = 4 * qc + 4
            nfl = len(fillers)
            for sweep in range(2):
                h0 = 2 * sweep
                avs = [ps.tile([128, 512], F32, tag="av", bufs=2,
                               name=f"avps{qc}_{sweep}_{hh}")
                       for hh in range(2)]
                pend = []

                def do_av(j, etp, avs=avs, h0=h0):
                    vs = max(0, 128 * (j - 4 * qc))
                    for hh in range(2):
                        nc.tensor.matmul(
                            avs[hh][0:65, vs:512],
                            lhsT=vg[:, j, h0 + hh, 0:65],
                            rhs=etp[:, hh, vs:512],
                            start=(j == 0), stop=(j == njt - 1),
                            skip_group_check=True)

                for j in range(njt):
                    vs = max(0, 128 * (j - 4 * qc))
                    scp = ps.tile([128, 2, 512], F32, tag="mm2", bufs=2)
                    for hh in range(2):
                        h = h0 + hh
                        hp = 64 * (h % 2)
                        gg = h // 2
                        nc.tensor.matmul(
                            scp[:, hh, vs:512],
                            lhsT=kt[hp:hp + 64, gg, 128 * j:128 * (j + 1)],
                            rhs=qt[hp:hp + 64, gg,
                                   512 * qc + vs:512 * (qc + 1)],
                            start=True, stop=True, skip_group_check=True)
                    etp = sb.tile([128, 2, 512], BF16, tag=f"et{sweep}",
                                  bufs=4)
                    nc.scalar.activation(etp[:, :, vs:512], scp[:, :, vs:512],
                                         EXP, scale=SCALE)
                    if j >= 4 * qc:
                        for hh in range(2):
                            nc.vector.tensor_mul(etp[:, hh, vs:vs + 128],
                                                 etp[:, hh, vs:vs + 128],
                                                 tri_t)
                    pend.append((j, etp))
                    if len(pend) > 2:   # lag-2 AV pipeline
                        do_av(*pend.pop(0))
                    slot = sweep * njt + j
                    k0 = nfl * slot // (2 * njt)
                    k1 = nfl * (slot + 1) // (2 * njt)
                    for k in range(k0, k1):
                        fillers[k]()
                for args in pend:
                    do_av(*args)
                # drain AV psum at the sweep boundary (frees the av banks)
                for hh in range(2):
                    raw = sb.tile([65, 512], F32R, tag="raw", bufs=8,
                                  name=f"raw{qc}_{h0 + hh}")
                    if qc == 3 and sweep == 1:
                        nc.scalar.activation(raw, avs[hh][0:65, :], COPY)
                    else:
                        nc.vector.tensor_copy(raw, avs[hh][0:65, :])
                    raw_of.setdefault(qc, []).append(raw)

        # ---- schedule ----
        # pre-phase: just enough projections for attention(0)
        qk_chain(wq_t, qt, 0, 0)
        qk_chain(wq_t, qt, 1, 0)
        qk_chain(wk_t, kt, 0, 0)
        qk_chain(wk_t, kt, 1, 0)
        for j in range(4):
            v_chain(j)

        attention(0, [
            lambda: qk_chain(wk_t, kt, 0, 1),
            lambda: qk_chain(wk_t, kt, 1, 1),
            lambda: qk_chain(wq_t, qt, 0, 1),
            lambda: qk_chain(wq_t, qt, 1, 1),
            lambda: v_chain(4),
            lambda: v_chain(5),
        ])
        attention(1, [
            lambda: v_chain(6),
            lambda: v_chain(7),
            lambda: qk_chain(wk_t, kt, 0, 2),
            lambda: qk_chain(wk_t, kt, 1, 2),
            lambda: part1_norm(0),
            lambda: qk_chain(wq_t, qt, 0, 2),
            lambda: qk_chain(wq_t, qt, 1, 2),
            lambda: v_chain(8),
            lambda: part2_unit(0, 0),
            lambda: part2_unit(0, 1),
            lambda: part2_unit(0, 2),
        ])
        attention(2, [
            lambda: v_chain(9),
            lambda: v_chain(10),
            lambda: v_chain(11),
            lambda: qk_chain(wk_t, kt, 0, 3),
            lambda: qk_chain(wk_t, kt, 1, 3),
            lambda: part2_unit(0, 3),
            lambda: part1_norm(1),
            lambda: qk_chain(wq_t, qt, 0, 3),
            lambda: qk_chain(wq_t, qt, 1, 3),
            lambda: part2_unit(1, 0),
            lambda: part2_unit(1, 1),
            lambda: v_chain(12),
        ])
        attention(3, [
            lambda: v_chain(13),
            lambda: v_chain(14),
            lambda: v_chain(15),
            lambda: part2_unit(1, 2),
            lambda: part2_unit(1, 3),
            lambda: part1_norm(2),
            lambda: part2_unit(2, 0),
            lambda: part2_unit(2, 1),
            lambda: part2_unit(2, 2),
            lambda: part2_unit(2, 3),
        ])
        # tail: normalize qc=3 in 128-column chunks so each O-projection
        # token tile starts as soon as its slice of the pairs is ready
        raws = raw_of[3]
        pairs = [sb.tile([128, 512], BF16, tag="pair", bufs=4,
                         name=f"pair3_{p}") for p in range(2)]
        rcps = []
        for h in range(NH):
            bc = ps.tile([64, 512], F32, tag="mm", bufs=2, name=f"bc3_{h}")
            nc.tensor.matmul(bc, lhsT=onesr_t[64:65, 0:64],
                             rhs=raws[h][64:65, :],
                             start=True, stop=True, skip_group_check=True)
            rcp = sb.tile([64, 512], F32, tag="rcp", bufs=4,
                          name=f"rcp3_{h}")
            nc.vector.reciprocal_approx_fast(rcp, bc)
            rcps.append(rcp)
        pair_of[3] = pairs
        for tt in range(4):
            cs = slice(128 * tt, 128 * (tt + 1))
            for h in range(NH):
                if h % 2 == 0:
                    nc.vector.tensor_mul(pairs[h // 2][0:64, cs],
                                         raws[h][0:64, cs], rcps[h][:, cs])
                else:
                    tmp = sb.tile([64, 128], BF16, tag="tmp3", bufs=4,
                                  name=f"tmp3_{h}_{tt}")
                    nc.vector.tensor_mul(tmp, raws[h][0:64, cs],
                                         rcps[h][:, cs])
                    nc.scalar.dma_start(pairs[h // 2][64:128, cs], tmp)
            part2_unit(3, tt)
    nc.compile()
    return nc


def _get_built():
    global _BUILT
    if _BUILT is None:
        _BUILT = _build()
    return _BUILT


def _host_inputs(x, q_proj, k_proj, v_proj, o_proj):
    bf = ml_dtypes.bfloat16
    # [p, tc, k, c] = x[b][512tc+c, 128k+p]
    xt = [np.ascontiguousarray(
        x[b].T.reshape(8, 128, 4, 512).transpose(1, 2, 0, 3).astype(bf))
        for b in range(B)]
    tri = np.triu(np.ones((128, 128), dtype=np.float32)).astype(bf)
    onesb = np.ones((128, 64), dtype=np.float32).astype(bf)
    onesr = np.ones((1, 64), dtype=np.float32)

    def wqk(w, g):
        # [fp, 1024*gg + 128*k + m] = w[256g+128gg+m, 128k+fp]
        a = w[256 * g:256 * (g + 1)].reshape(2, 128, 8, 128)
        return np.ascontiguousarray(
            a.transpose(3, 0, 2, 1).reshape(128, 2048).astype(bf))

    def wvt(w, g):
        # [fp, 256*k + vd] = w[256g+vd, 128k+fp]
        a = w[256 * g:256 * (g + 1)].reshape(256, 8, 128)
        return np.ascontiguousarray(
            a.transpose(2, 1, 0).reshape(128, 2048).astype(bf))

    def wo_s(w, g):
        # [dp, 1024*dd + o] = o_proj[o, 256g+128dd+dp]
        a = w[:, 256 * g:256 * (g + 1)].reshape(1024, 2, 128)
        return np.ascontiguousarray(
            a.transpose(2, 1, 0).reshape(128, 2048).astype(bf))

    in_maps = []
    for c in range(NCORES):
        b, g = c // 4, c % 4
        in_maps.append(dict(
            xt=xt[b], wq=wqk(q_proj, g), wk=wqk(k_proj, g),
            wvt=wvt(v_proj, g), wo=wo_s(o_proj, g), tri=tri,
            onesb=onesb, onesr=onesr))
    return in_maps


def kernel(**inputs):
    x = np.asarray(inputs["x"], dtype=np.float32)
    q_proj = np.asarray(inputs["q_proj"], dtype=np.float32)
    k_proj = np.asarray(inputs["k_proj"], dtype=np.float32)
    v_proj = np.asarray(inputs["v_proj"], dtype=np.float32)
    o_proj = np.asarray(inputs["o_proj"], dtype=np.float32)

    in_maps = _host_inputs(x, q_proj, k_proj, v_proj, o_proj)
    nc = _get_built()
    global LAST_RESULTS
    LAST_RESULTS = run_bass_kernel_spmd(
        nc, in_maps, core_ids=list(range(NCORES)),
        trace=bool(os.environ.get("KERNEL_TRACE")))
    out = np.zeros((B, S, D), dtype=np.float32)
    for c in range(NCORES):
        out[c // 4] += np.asarray(
            LAST_RESULTS.results[c]["out"]).astype(np.float32)
    return out


# revision 35
# speedup vs baseline: 1.0179x; 1.0114x over previous
"""Multi-head self-attention (B=2, S=2048, D=1024, H=16, causal) on 8 trn2 cores.

Sharding: core c = (batch b=c//4, head-group g=c%4 of 4 heads = dims
256g:256g+256). Column-parallel QKV, row-parallel O; each core returns a
partial [2048, 1024] output for its batch; host sums 4 partials per batch.

Per-core kernel (all matmul inputs bf16):
  - projections from xt [1024, 2048] bf16: QT/KT [128, 2, 2048] bf16;
    V computed directly in token-major layout vg[128, 16, 4, 65] with a
    ones column per head so AV also produces the softmax denominator.
  - attention in transposed-score layout: scoresT[k, q] = K @ Q^T tiles,
    exp on ACT (scale 1/8 fused) to bf16, causal staircase skips invalid
    columns, triangular mask on diagonal blocks only. Lag-2 AV pipeline.
  - software pipeline: only the first token-chunk of Q/K and V tiles 0-3
    are projected up front; the remaining projection chains run as PE
    fillers inside the attention j-loops (which are paced by the exp
    stream on ACT), as do the deferred normalize + O-projection units.
  - attention runs two sweeps of 2 heads per q-chunk so the exp covers
    a [128, 2, 512] two-bank PSUM pair in one ACT instruction (halves
    exp instruction count and ACT semaphore waits).
  - normalize: raw AV copy to SBUF (frees PSUM), denominator broadcast
    via f32r ones outer-product matmul, reciprocal_approx_fast on DVE
    (no Ln -> no ACT table switches), multiply to bf16 pairs; odd heads
    DMA-shifted to partitions 64:128 so O contracts 128 dims per pair.
  - O projection: 2 chained matmuls per 512-col chunk, single output
    DMA per 128-token tile; the qc=3 normalize is chunked by token tile
    so the tail O units start as early as possible.
"""

import os
import numpy as np
from contextlib import ExitStack

import ml_dtypes

import concourse.bass as bass
import concourse.tile as tile
from concourse import bacc, mybir
from concourse.bass_utils import run_bass_kernel_spmd

F32R = mybir.dt.float32r
F32 = mybir.dt.float32
BF16 = mybir.dt.bfloat16
EXP = mybir.ActivationFunctionType.Exp
COPY = mybir.ActivationFunctionType.Copy

B, S, D = 2, 2048, 1024
NCORES = 8
SCALE = 0.125         # 1/sqrt(64)
NH = 4                # heads per core

_BUILT = None
LAST_RESULTS = None


def _build():
    nc = bacc.Bacc("TRN2", target_bir_lowering=False, debug=False,
                   num_devices=NCORES)
    # xt packed [p, tc, k, c]: xt[p, tc, k, c] = x[b][512tc+c, 128k+p]
    xt_d = nc.dram_tensor("xt", [128, 4, 8, 512], BF16,
                          kind="ExternalInput").ap()
    wq_d = nc.dram_tensor("wq", [128, 2048], BF16, kind="ExternalInput").ap()
    wk_d = nc.dram_tensor("wk", [128, 2048], BF16, kind="ExternalInput").ap()
    wvt_d = nc.dram_tensor("wvt", [128, 2048], BF16, kind="ExternalInput").ap()
    wo_d = nc.dram_tensor("wo", [128, 2048], BF16, kind="ExternalInput").ap()
    tri_d = nc.dram_tensor("tri", [128, 128], BF16, kind="ExternalInput").ap()
    onesb_d = nc.dram_tensor("onesb", [128, 64], BF16,
                             kind="ExternalInput").ap()
    onesr_d = nc.dram_tensor("onesr", [1, 64], F32R, kind="ExternalInput").ap()
    out_d = nc.dram_tensor("out", [S, D], BF16, kind="ExternalOutput").ap()

    with tile.TileContext(nc) as tc, ExitStack() as ctx:
        consts = ctx.enter_context(tc.tile_pool(name="consts", bufs=1))
        sb = ctx.enter_context(tc.tile_pool(name="sb", bufs=1))
        ps = ctx.enter_context(tc.tile_pool(name="ps", bufs=1, space="PSUM"))

        # ---- DMA program: weights on the ACT hwdge queue, x on sync ----
        # (halves so the first projection chains start sooner)
        wq_t = consts.tile([128, 2048], BF16, tag="wq")
        nc.scalar.dma_start(wq_t[:, 0:256], wq_d[:, 0:256])
        nc.scalar.dma_start(wq_t[:, 256:1024], wq_d[:, 256:1024])
        nc.scalar.dma_start(wq_t[:, 1024:2048], wq_d[:, 1024:2048])
        wk_t = consts.tile([128, 2048], BF16, tag="wk")
        nc.scalar.dma_start(wk_t[:, 0:1024], wk_d[:, 0:1024])
        nc.scalar.dma_start(wk_t[:, 1024:2048], wk_d[:, 1024:2048])
        wvt_t = consts.tile([128, 2048], BF16, tag="wvt")
        nc.scalar.dma_start(wvt_t, wvt_d)
        wo_t = consts.tile([128, 2048], BF16, tag="wo")
        nc.scalar.dma_start(wo_t, wo_d)
        # x: one big contiguous DMA per token-chunk (tc0 split for latency)
        xts = []
        for tc2 in range(4):
            xk = sb.tile([128, 8, 512], BF16, tag="xts", bufs=4,
                         name=f"xts{tc2}")
            if tc2 == 0:
                for kq in range(4):
                    nc.sync.dma_start(xk[:, 2 * kq:2 * kq + 2, :],
                                      xt_d[:, 0, 2 * kq:2 * kq + 2, :])
            else:
                nc.sync.dma_start(xk, xt_d[:, tc2, :, :])
            xts.append(xk)
        tri_t = consts.tile([128, 128], BF16, tag="tri")
        nc.gpsimd.dma_start(tri_t, tri_d)
        onesb_t = consts.tile([128, 64], BF16, tag="onesb")
        nc.gpsimd.dma_start(onesb_t, onesb_d)
        # ones row at partition 64 (same base partition as the den row)
        onesr_t = consts.tile([65, 64], F32R, tag="onesr")
        nc.gpsimd.dma_start(onesr_t[64:65, :], onesr_d)

        # DVFS warm-up: dummy matmuls on uninitialized SBUF start right
        # after the SPMD barrier (no DMA dependency) so the PE clock is
        # ramped before the first real projection chain's data lands.
        warm = sb.tile([128, 512], F32, tag="warm")
        nc.vector.memset(warm, 0.0)
        for _ in range(2):
            wp = ps.tile([128, 512], F32, tag="mm", bufs=2, name="warmup")
            nc.tensor.matmul(wp, lhsT=warm[:, 0:128], rhs=warm,
                             start=True, stop=True, skip_group_check=True)

        qt = sb.tile([128, 2, 2048], BF16, tag="qt")
        kt = sb.tile([128, 2, 2048], BF16, tag="kt")
        vg = sb.tile([128, 16, NH, 65], BF16, tag="vg")
        # strided ones fill on the Pool engine (a strided DMA here would
        # degenerate to 2-byte SWDGE packets)
        nc.gpsimd.tensor_copy(vg[:, :, :, 64:65], onesb_t[:, 0:64])

        def qk_chain(w_t, dst, gg, tc2):
            pp = ps.tile([128, 512], F32, tag="mm", bufs=2)
            for k in range(8):
                nc.tensor.matmul(
                    pp,
                    lhsT=w_t[:, 1024 * gg + 128 * k:1024 * gg + 128 * (k + 1)],
                    rhs=xts[tc2][:, k, :],
                    start=(k == 0), stop=(k == 7), skip_group_check=True)
            nc.vector.tensor_copy(dst[:, gg, 512 * tc2:512 * (tc2 + 1)], pp)

        def v_chain(j):
            vp = ps.tile([128, 256], F32, tag="mm", bufs=2)
            for k in range(8):
                nc.tensor.matmul(
                    vp,
                    lhsT=xts[j // 4][:, k, 128 * (j % 4):128 * (j % 4 + 1)],
                    rhs=wvt_t[:, 256 * k:256 * (k + 1)],
                    start=(k == 0), stop=(k == 7), skip_group_check=True)
            nc.vector.tensor_copy(vg[:, j, :, 0:64], vp)

        # ---- deferred normalize (part1) / O projection (part2) ----
        raw_of = {}
        pair_of = {}

        def part1_norm(qc):
            raws = raw_of[qc]
            pairs = [sb.tile([128, 512], BF16, tag="pair", bufs=4,
                             name=f"pair{qc}_{p}") for p in range(2)]
            for h in range(NH):
                bc = ps.tile([64, 512], F32, tag="mm", bufs=2,
                             name=f"bc{qc}_{h}")
                nc.tensor.matmul(bc, lhsT=onesr_t[64:65, 0:64],
                                 rhs=raws[h][64:65, :],
                                 start=True, stop=True,
                                 skip_group_check=True)
                rcp = sb.tile([64, 512], F32, tag="rcp", bufs=4,
                              name=f"rcp{qc}_{h}")
                nc.vector.reciprocal_approx_fast(rcp, bc)
                if h % 2 == 0:
                    nc.vector.tensor_mul(pairs[h // 2][0:64, :],
                                         raws[h][0:64, :], rcp)
                else:
                    tmp = sb.tile([64, 512], BF16, tag="tmp", bufs=2,
                                  name=f"tmp{qc}_{h}")
                    nc.vector.tensor_mul(tmp, raws[h][0:64, :], rcp)
                    nc.scalar.dma_start(pairs[h // 2][64:128, :], tmp)
            pair_of[qc] = pairs

        def part2_unit(qc, tt):
            pairs = pair_of[qc]
            row0 = 512 * qc + 128 * tt
            ost = sb.tile([128, 1024], BF16, tag="ost", bufs=3,
                          name=f"ost{qc}_{tt}")
            for chv in range(2):
                op = ps.tile([128, 512], F32, tag="mm", bufs=2,
                             name=f"op{qc}_{tt}_{chv}")
                nc.tensor.matmul(
                    op, lhsT=pairs[0][:, 128 * tt:128 * (tt + 1)],
                    rhs=wo_t[:, 512 * chv:512 * (chv + 1)],
                    start=True, stop=False, skip_group_check=True)
                nc.tensor.matmul(
                    op, lhsT=pairs[1][:, 128 * tt:128 * (tt + 1)],
                    rhs=wo_t[:, 1024 + 512 * chv:1024 + 512 * (chv + 1)],
                    start=False, stop=True, skip_group_check=True)
                if chv == 0:
                    nc.vector.tensor_copy(ost[:, 0:512], op)
                else:
                    nc.scalar.activation(ost[:, 512:1024], op, COPY)
            nc.sync.dma_start(out_d[row0:row0 + 128, :], ost)

        # ---- attention: two sweeps of 2 heads; 1024-wide exp per j ----
        def attention(qc, fillers):
            njt = 4 * qc + 4
            nfl = len(fillers)
            for sweep in range(2):
                h0 = 2 * sweep
                avs = [ps.tile([128, 512], F32, tag="av", bufs=2,
                               name=f"avps{qc}_{sweep}_{hh}")
                       for hh in range(2)]
                pend = []

                def do_av(j, etp, avs=avs, h0=h0):
                    vs = max(0, 128 * (j - 4 * qc))
                    for hh in range(2):
                        nc.tensor.matmul(
                            avs[hh][0:65, vs:512],
                            lhsT=vg[:, j, h0 + hh, 0:65],
                            rhs=etp[:, hh, vs:512],
                            start=(j == 0), stop=(j == njt - 1),
                            skip_group_check=True)

                for j in range(njt):
                    vs = max(0, 128 * (j - 4 * qc))
                    scp = ps.tile([128, 2, 512], F32, tag="mm2", bufs=2)
                    for hh in range(2):
                        h = h0 + hh
                        hp = 64 * (h % 2)
                        gg = h // 2
                        nc.tensor.matmul(
                            scp[:, hh, vs:512],
                            lhsT=kt[hp:hp + 64, gg, 128 * j:128 * (j + 1)],
                            rhs=qt[hp:hp + 64, gg,
                                   512 * qc + vs:512 * (qc + 1)],
                            start=True, stop=True, skip_group_check=True)
                    etp = sb.tile([128, 2, 512], BF16, tag=f"et{sweep}",
                                  bufs=4)
                    nc.scalar.activation(etp[:, :, vs:512], scp[:, :, vs:512],
                                         EXP, scale=SCALE)
                    if j >= 4 * qc:
                        for hh in range(2):
                            nc.vector.tensor_mul(etp[:, hh, vs:vs + 128],
                                                 etp[:, hh, vs:vs + 128],
                                                 tri_t)
                    pend.append((j, etp))
                    if len(pend) > 2:   # lag-2 AV pipeline
                        do_av(*pend.pop(0))
                    slot = sweep * njt + j
                    k0 = nfl * slot // (2 * njt)
                    k1 = nfl * (slot + 1) // (2 * njt)
                    for k in range(k0, k1):
                        fillers[k]()
                for args in pend:
                    do_av(*args)
                # drain AV psum at the sweep boundary (frees the av banks)
                for hh in range(2):
                    raw = sb.tile([65, 512], F32R, tag="raw", bufs=8,
                                  name=f"raw{qc}_{h0 + hh}")
                    if qc == 3 and sweep == 1:
                        nc.scalar.activation(raw, avs[hh][0:65, :], COPY)
                    else:
                        nc.vector.tensor_copy(raw, avs[hh][0:65, :])
                    raw_of.setdefault(qc, []).append(raw)

        # ---- schedule ----
        # pre-phase: just enough projections for attention(0)
        qk_chain(wq_t, qt, 0, 0)
        qk_chain(wq_t, qt, 1, 0)
        qk_chain(wk_t, kt, 0, 0)
        qk_chain(wk_t, kt, 1, 0)
        for j in range(4):
            v_chain(j)

        attention(0, [
            lambda: qk_chain(wk_t, kt, 0, 1),
            lambda: qk_chain(wk_t, kt, 1, 1),
            lambda: qk_chain(wq_t, qt, 0, 1),
            lambda: qk_chain(wq_t, qt, 1, 1),
            lambda: v_chain(4),
            lambda: v_chain(5),
        ])
        attention(1, [
            lambda: v_chain(6),
            lambda: v_chain(7),
            lambda: qk_chain(wk_t, kt, 0, 2),
            lambda: qk_chain(wk_t, kt, 1, 2),
            lambda: part1_norm(0),
            lambda: qk_chain(wq_t, qt, 0, 2),
            lambda: qk_chain(wq_t, qt, 1, 2),
            lambda: v_chain(8),
            lambda: part2_unit(0, 0),
            lambda: part2_unit(0, 1),
            lambda: part2_unit(0, 2),
        ])
        attention(2, [
            lambda: v_chain(9),
            lambda: v_chain(10),
            lambda: v_chain(11),
            lambda: qk_chain(wk_t, kt, 0, 3),
            lambda: qk_chain(wk_t, kt, 1, 3),
            lambda: part2_unit(0, 3),
            lambda: part1_norm(1),
            lambda: qk_chain(wq_t, qt, 0, 3),
            lambda: qk_chain(wq_t, qt, 1, 3),
            lambda: part2_unit(1, 0),
            lambda: part2_unit(1, 1),
            lambda: v_chain(12),
        ])
        attention(3, [
            lambda: v_chain(13),
            lambda: v_chain(14),
            lambda: v_chain(15),
            lambda: part2_unit(1, 2),
            lambda: part2_unit(1, 3),
            lambda: part1_norm(2),
            lambda: part2_unit(2, 0),
            lambda: part2_unit(2, 1),
            lambda: part2_unit(2, 2),
            lambda: part2_unit(2, 3),
        ])
        # tail: normalize qc=3 in 128-column chunks so each O-projection
        # token tile starts as soon as its slice of the pairs is ready
        raws = raw_of[3]
        pairs = [sb.tile([128, 512], BF16, tag="pair", bufs=4,
                         name=f"pair3_{p}") for p in range(2)]
        rcps = []
        for h in range(NH):
            bc = ps.tile([64, 512], F32, tag="mm", bufs=2, name=f"bc3_{h}")
            nc.tensor.matmul(bc, lhsT=onesr_t[64:65, 0:64],
                             rhs=raws[h][64:65, :],
                             start=True, stop=True, skip_group_check=True)
            rcp = sb.tile([64, 512], F32, tag="rcp", bufs=4,
                          name=f"rcp3_{h}")
            nc.vector.reciprocal_approx_fast(rcp, bc)
            rcps.append(rcp)
        pair_of[3] = pairs
        for tt in range(4):
            cs = slice(128 * tt, 128 * (tt + 1))
            for h in range(NH):
                if h % 2 == 0:
                    nc.vector.tensor_mul(pairs[h // 2][0:64, cs],
                                         raws[h][0:64, cs], rcps[h][:, cs])
                else:
                    tmp = sb.tile([64, 128], BF16, tag="tmp3", bufs=4,
                                  name=f"tmp3_{h}_{tt}")
                    nc.vector.tensor_mul(tmp, raws[h][0:64, cs],
                                         rcps[h][:, cs])
                    nc.scalar.dma_start(pairs[h // 2][64:128, cs], tmp)
            part2_unit(3, tt)
    nc.compile()
    return nc


def _get_built():
    global _BUILT
    if _BUILT is None:
        _BUILT = _build()
    return _BUILT


def _host_inputs(x, q_proj, k_proj, v_proj, o_proj):
    bf = ml_dtypes.bfloat16
    # [p, tc, k, c] = x[b][512tc+c, 128k+p]
    xt = [np.ascontiguousarray(
        x[b].T.reshape(8, 128, 4, 512).transpose(1, 2, 0, 3).astype(bf))
        for b in range(B)]
    tri = np.triu(np.ones((128, 128), dtype=np.float32)).astype(bf)
    onesb = np.ones((128, 64), dtype=np.float32).astype(bf)
    onesr = np.ones((1, 64), dtype=np.float32)

    def wqk(w, g):
        # [fp, 1024*gg + 128*k + m] = w[256g+128gg+m, 128k+fp]
        a = w[256 * g:256 * (g + 1)].reshape(2, 128, 8, 128)
        return np.ascontiguousarray(
            a.transpose(3, 0, 2, 1).reshape(128, 2048).astype(bf))

    def wvt(w, g):
        # [fp, 256*k + vd] = w[256g+vd, 128k+fp]
        a = w[256 * g:256 * (g + 1)].reshape(256, 8, 128)
        return np.ascontiguousarray(
            a.transpose(2, 1, 0).reshape(128, 2048).astype(bf))

    def wo_s(w, g):
        # [dp, 1024*dd + o] = o_proj[o, 256g+128dd+dp]
        a = w[:, 256 * g:256 * (g + 1)].reshape(1024, 2, 128)
        return np.ascontiguousarray(
            a.transpose(2, 1, 0).reshape(128, 2048).astype(bf))

    in_maps = []
    for c in range(NCORES):
        b, g = c // 4, c % 4
        in_maps.append(dict(
            xt=xt[b], wq=wqk(q_proj, g), wk=wqk(k_proj, g),
            wvt=wvt(v_proj, g), wo=wo_s(o_proj, g), tri=tri,
            onesb=onesb, onesr=onesr))
    return in_maps


def kernel(**inputs):
    x = np.asarray(inputs["x"], dtype=np.float32)
    q_proj = np.asarray(inputs["q_proj"], dtype=np.float32)
    k_proj = np.asarray(inputs["k_proj"], dtype=np.float32)
    v_proj = np.asarray(inputs["v_proj"], dtype=np.float32)
    o_proj = np.asarray(inputs["o_proj"], dtype=np.float32)

    in_maps = _host_inputs(x, q_proj, k_proj, v_proj, o_proj)
    nc = _get_built()
    global LAST_RESULTS
    LAST_RESULTS = run_bass_kernel_spmd(
        nc, in_maps, core_ids=list(range(NCORES)),
        trace=bool(os.environ.get("KERNEL_TRACE")))
    out = np.zeros((B, S, D), dtype=np.float32)
    for c in range(NCORES):
        out[c // 4] += np.asarray(
            LAST_RESULTS.results[c]["out"]).astype(np.float32)
    return out
